# revision 18
# baseline (speedup 1.0000x reference)
"""Distributed Trainium2 kernel for the 21-qubit staircase variational circuit.

Math: the circuit is (RY encoding + Rot layer + CNOT chain) x 3 + <Z_w>.
Each CNOT chain is a computational-basis permutation (prefix-XOR), so the
state just before the FINAL chain decomposes exactly, per 8-way shard on
wires 0..2 (most-significant), as a rank-4 sum of outer products
    psi^{(d)}[p, f] = sum_{t<4} U_t[d, p] * W_t[f]
with U_t complex [8,128] (wires 3..9) and W_t complex [2048] (wires 10..20).
The final chain folds into prefix-parity observables
    <Z_w>_final = sum_b |psi[b]|^2 * (-1)^(b_0^...^b_w).

Host does only O(2^11) preprocessing of these small vectors. Each NeuronCore
materializes its 2^18-amplitude shard (rank-8 real matmul), squares it into
probabilities on the Act engine, contracts the partition-sign masks back on
the PE, and applies the f-axis sign masks + reduction on the DVE — the
memory-bound part. Host applies the 8 per-core +-1 weights (SD) and sums.

Device schedule (TimelineSim: 11971 ns/core vs 33550 ns for the v1 kernel):
  * bf16 matmul inputs: 1 PE cycle/row (vs 4 for fp32), half the DMA bytes.
    Error budget: amplitudes are ~2^-10 rms, so bf16's 2^-9 relative rounding
    noise cancels over the 2^18-element signed reductions (measured rel err
    ~1e-3 vs the 2e-2 gate).
  * One fused [8,4224] chunk-major input strip (uu | wre_q wim_q per chunk);
    the first DMA carries only chunk 0 so the amplitude matmuls start at
    t~3.3us, bounded by the fixed DMA dispatch+semaphore latency.
  * A dummy warm-up matmul on a memset row starts the PE clock ramp early,
    so all real matmuls run at the full p-state (213 ns per 512-col matmul).
  * Per 512-col f-chunk: 2 amplitude matmuls (re|im) into a 2-bank PSUM
    tile, one 1024-col Act Square into a per-chunk bf16 SBUF slice (no
    buffer-reuse waits on Act: its 4 ops run back-to-back, the critical
    4.15us of the pipeline), 2 accumulate obs matmuls [128->21], one DVE
    scalar_tensor_tensor with the sign mask and row-accumulate.
  * PE order amp0,amp1,amp2,obs0,amp3,obs1,obs2,obs3 keeps Act gapless.
  * Output [21,4] (per-chunk partials) leaves via one SP DMA; SP spins on
    the completion semaphore (dropping it can crash the exec unit with
    NRT_EXEC_UNIT_UNRECOVERABLE when the NEFF retires mid-DMA). The host
    sums the 4 chunk columns.

Toolchain constraints found the hard way (walrus/HW, not caught by CoreSim):
matmul PSUM writes must start at a bank boundary; DVE ops may read at most
one PSUM operand; InstTriggerDma does not survive raw-walrus codegen (so a
pre-armed SWDGE scatter-add output path is not available here).
"""
import numpy as np
import ml_dtypes

N = 21
ND, NP, NF = 3, 7, 11

# ----------------------------------------------------------------------------
# host-side small-vector math
# ----------------------------------------------------------------------------


def _ry_v(theta):
    return np.array([np.cos(0.5 * theta), np.sin(0.5 * theta)], dtype=np.complex128)


def _rot_m(phi, theta, omega):
    c, s = np.cos(0.5 * theta), np.sin(0.5 * theta)
    return np.array(
        [
            [np.exp(-0.5j * (phi + omega)) * c, -np.exp(0.5j * (phi - omega)) * s],
            [np.exp(-0.5j * (phi - omega)) * s, np.exp(0.5j * (phi + omega)) * c],
        ],
        dtype=np.complex128,
    )


def _bits(nbits):
    idx = np.arange(1 << nbits)
    return [(idx >> (nbits - 1 - i)) & 1 for i in range(nbits)]


def _chain_vec(vs, prev_bit, nbits):
    bits = _bits(nbits)
    out = np.ones(1 << nbits, np.complex128)
    prev = np.full(1 << nbits, prev_bit)
    for i, v in enumerate(vs):
        out = out * v[bits[i] ^ prev]
        prev = bits[i]
    return out


def _chain_src_idx(nbits, prev_bit):
    bits = _bits(nbits)
    src = np.zeros(1 << nbits, np.int64)
    prev = np.full(1 << nbits, prev_bit)
    for i in range(nbits):
        src = (src << 1) | (bits[i] ^ prev)
        prev = bits[i]
    return src


def _apply_1q(vecs, gate, bit, nbits):
    lead = vecs.shape[:-1]
    a = vecs.reshape(lead + (1 << bit, 2, -1))
    out = np.einsum("ab,...bq->...aq", gate, a)
    return out.reshape(lead + (1 << nbits,))


def build_terms(x, params):
    x = np.asarray(x, np.float64)
    params = np.asarray(params, np.float64)
    v = [np.asarray(_rot_m(*params[0, w]) @ _ry_v(x[w])) for w in range(N)]

    U = np.zeros((2, 8, 128), np.complex128)
    W = np.zeros((2, 2048), np.complex128)
    par_p = np.arange(128) & 1
    for d in range(8):
        c0, c1, c2 = (d >> 2) & 1, (d >> 1) & 1, d & 1
        alpha = v[0][c0] * v[1][c0 ^ c1] * v[2][c1 ^ c2]
        A = _chain_vec([v[w] for w in range(3, 10)], c2, NP)
        U[0, d] = alpha * A * (par_p == 0)
        U[1, d] = alpha * A * (par_p == 1)
    W[0] = _chain_vec([v[w] for w in range(10, 21)], 0, NF)
    W[1] = _chain_vec([v[w] for w in range(10, 21)], 1, NF)

    def apply_layer(U, W, r):
        g = [_rot_m(*params[r, w]) for w in range(N)]
        for w in range(10, 21):
            W = _apply_1q(W, g[w], w - 10, NF)
        for w in range(3, 10):
            U = _apply_1q(U, g[w], w - 3, NP)
        G8 = np.kron(g[0], np.kron(g[1], g[2]))
        U = np.einsum("de,ten->tdn", G8, U)
        return U, W

    U, W = apply_layer(U, W, 1)

    T = U.shape[0]
    Un = np.zeros((2 * T, 8, 128), np.complex128)
    Wn = np.zeros((2 * T, 2048), np.complex128)
    srcf = [_chain_src_idx(NF, s) for s in (0, 1)]
    for d in range(8):
        c0, c1, c2 = (d >> 2) & 1, (d >> 1) & 1, d & 1
        md = (c0 << 2) | ((c0 ^ c1) << 1) | (c1 ^ c2)
        srcp = _chain_src_idx(NP, c2)
        for t in range(T):
            base = U[t, md][srcp]
            for s in (0, 1):
                Un[2 * t + s, d] = base * (par_p == s)
    for t in range(T):
        for s in (0, 1):
            Wn[2 * t + s] = W[t][srcf[s]]
    return apply_layer(Un, Wn, 2)


def sign_tables():
    pbits = np.array(_bits(NP)).T
    fbits = np.array(_bits(NF)).T
    dbits = np.array(_bits(ND)).T
    SA = np.ones((128, N), np.float32)
    SF = np.ones((N, 2048), np.float32)
    SD = np.ones((8, N), np.float32)
    for w in range(N):
        if w <= 2:
            SD[:, w] = (-1.0) ** (dbits[:, : w + 1].sum(1))
        elif w <= 9:
            SD[:, w] = (-1.0) ** (dbits.sum(1))
            SA[:, w] = (-1.0) ** (pbits[:, : w - 2].sum(1))
        else:
            SD[:, w] = (-1.0) ** (dbits.sum(1))
            SA[:, w] = (-1.0) ** (pbits.sum(1))
            SF[w, :] = (-1.0) ** (fbits[:, : w - 9].sum(1))
    return SA, SF, SD


# ----------------------------------------------------------------------------
# device kernel
# ----------------------------------------------------------------------------
_NC_CACHE = {}

_BF16 = ml_dtypes.bfloat16


def _build_nc():
    import sys

    from contextlib import ExitStack  # noqa: F401  (used by builder)

    # the builder lives inline here so kernel.py stays self-contained
    return _build_nc_v5()


def _build_nc_v5():
    import concourse.bass as bass
    import concourse.mybir as mybir
    from contextlib import ExitStack

    f32 = mybir.dt.float32
    bf16 = mybir.dt.bfloat16
    nc = bass.Bass()
    big_d = nc.declare_dram_parameter("big", [8, 4224], bf16, isOutput=False)
    sa_d = nc.declare_dram_parameter("sa", [128, N], bf16, isOutput=False)
    sf_d = nc.declare_dram_parameter("sf", [N, 2048], bf16, isOutput=False)
    out_d = nc.declare_dram_parameter("out", [N, 4], f32, isOutput=True)
    SQ = mybir.ActivationFunctionType.Square
    MUL = mybir.AluOpType.mult

    with ExitStack() as stack:
        ec = stack.enter_context
        big_t = ec(nc.sbuf_tensor("big_t", [8, 4224], bf16))
        sa_t = ec(nc.sbuf_tensor("sa_t", [128, N], bf16))
        sf_t = ec(nc.sbuf_tensor("sf_t", [N, 2048], bf16))
        warm_t = ec(nc.sbuf_tensor("warm_t", [1, 256], bf16))
        sq_t = ec(nc.sbuf_tensor("sq_t", [128, 4096], bf16))
        scratch = ec(nc.sbuf_tensor("scratch", [N, 2048], f32))
        res_t = ec(nc.sbuf_tensor("res_t", [N, 4], f32))
        warm_p = ec(nc.psum_tensor("warm_p", [1, 256], f32))
        pp0 = ec(nc.psum_tensor("pp0", [128, 1024], f32))
        pp1 = ec(nc.psum_tensor("pp1", [128, 1024], f32))
        po0 = ec(nc.psum_tensor("po0", [N, 512], f32))
        po1 = ec(nc.psum_tensor("po1", [N, 512], f32))
        block = ec(nc.Block())
        s_big0 = ec(nc.semaphore("s_big0"))
        s_big = ec(nc.semaphore("s_big"))
        s_sa = ec(nc.semaphore("s_sa"))
        s_sf = ec(nc.semaphore("s_sf"))
        s_w = ec(nc.semaphore("s_w"))
        s_mm = ec(nc.semaphore("s_mm"))
        s_sqa = ec(nc.semaphore("s_sqa"))
        s_obs = ec(nc.semaphore("s_obs"))
        s_red = ec(nc.semaphore("s_red"))
        s_out = ec(nc.semaphore("s_out"))
        pp = (pp0, pp1)
        po = (po0, po1)

        def wre_v(q):
            return big_t[:, 128 + 1024 * q : 640 + 1024 * q]

        def wim_v(q):
            return big_t[:, 640 + 1024 * q : 1152 + 1024 * q]

        def sync_body(sync):
            sync.dma_start(out=big_t[:, 0:1152], in_=big_d[:, 0:1152]).then_inc(s_big0, 16)
            sync.dma_start(out=big_t[:, 1152:4224], in_=big_d[:, 1152:4224]).then_inc(s_big, 16)
            sync.dma_start(out=sa_t[:], in_=sa_d[:]).then_inc(s_sa, 16)
            sync.dma_start(out=sf_t[:], in_=sf_d[:]).then_inc(s_sf, 16)
            sync.wait_ge(s_red, 4)
            sync.dma_start(out=out_d[:], in_=res_t[:]).then_inc(s_out, 16)
            sync.wait_ge(s_out, 16)

        block.sync(sync_body)

        def wait_sq(te, q):
            te.wait_ge(s_sqa, q + 1)

        def amp(te, q):
            b = q % 2
            te.matmul(pp[b][:, 0:512], big_t[:, 0:128], wre_v(q), start=True, stop=True)
            te.matmul(
                pp[b][:, 512:1024], big_t[:, 0:128], wim_v(q), start=True, stop=True
            ).then_inc(s_mm, 1)

        def obs(te, q):
            b = q % 2
            te.matmul(po[b][:], sa_t[:], sq_t[:, 1024 * q : 1024 * q + 512], start=True, stop=False)
            te.matmul(po[b][:], sa_t[:], sq_t[:, 1024 * q + 512 : 1024 * q + 1024], start=False, stop=True).then_inc(s_obs, 1)

        def tensor_body(te):
            # warm-up matmul starts the PE clock ramp early
            te.wait_ge(s_w, 1)
            te.matmul(warm_p[:], warm_t[0:1, 0:1], warm_t[0:1, :], start=True, stop=True)
            te.wait_ge(s_big0, 16)
            amp(te, 0)
            te.wait_ge(s_big, 16)
            amp(te, 1)
            wait_sq(te, 0)
            amp(te, 2)
            te.wait_ge(s_sa, 16)
            obs(te, 0)
            wait_sq(te, 1)
            amp(te, 3)
            obs(te, 1)
            wait_sq(te, 2)
            te.wait_ge(s_red, 1)
            obs(te, 2)
            wait_sq(te, 3)
            te.wait_ge(s_red, 2)
            obs(te, 3)

        block.tensor(tensor_body)

        def scalar_body(sc):
            for q in range(4):
                b = q % 2
                sc.wait_ge(s_mm, q + 1)
                sc.activation(
                    sq_t[:, 1024 * q : 1024 * (q + 1)],
                    pp[b][:, 0:1024],
                    func=SQ,
                ).then_inc(s_sqa, 1)

        block.scalar(scalar_body)

        def vector_body(v):
            v.memset(warm_t[0:1, :], 0.0).then_inc(s_w, 1)

            def stt(q):
                b = q % 2
                if q == 0:
                    v.wait_ge(s_sf, 16)
                v.wait_ge(s_obs, q + 1)
                v.scalar_tensor_tensor(
                    out=scratch[:, 512 * q : 512 * (q + 1)],
                    in0=po[b][:],
                    scalar=1.0,
                    in1=sf_t[:, 512 * q : 512 * (q + 1)],
                    op0=MUL,
                    op1=MUL,
                    accum_out=res_t[:, q : q + 1],
                ).then_inc(s_red, 1)

            stt(0)
            stt(1)
            stt(2)
            stt(3)

        block.vector(vector_body)


    return nc


def make_in_maps(x, params):
    """Host preprocessing -> per-core input dicts for run_bass_kernel_spmd."""
    U, W = build_terms(x, params)  # U [4,8,128] complex, W [4,2048] complex
    SA, SF, SD = sign_tables()

    wre = np.concatenate([W.real, -W.imag])  # [8, 2048]
    wim = np.concatenate([W.imag, W.real])  # [8, 2048]
    sa = np.ascontiguousarray(SA.astype(_BF16))
    sf = np.ascontiguousarray(SF.astype(_BF16))

    in_maps = []
    for d in range(8):
        uu = np.concatenate([U[:, d].real, U[:, d].imag])  # [8, 128]
        # chunk-major strip: [uu | wre0 wim0 | wre1 wim1 | wre2 wim2 | wre3 wim3]
        parts = [uu]
        for q in range(4):
            parts.append(wre[:, 512 * q : 512 * (q + 1)])
            parts.append(wim[:, 512 * q : 512 * (q + 1)])
        big = np.concatenate(parts, axis=1).astype(_BF16)
        in_maps.append(
            {
                "big": np.ascontiguousarray(big),
                "sa": sa,
                "sf": sf,
            }
        )
    return in_maps, SD


def _get_runner():
    """Build the PJRT executable once and cache the jitted callable.

    Mirrors concourse.bass2jax.run_bass_via_pjrt, which reconstructs (and
    re-traces) the jax.jit closure on every call (~200 ms of host overhead
    per invocation). Falls back to the stock path on any mismatch.
    """
    if "runner" in _NC_CACHE:
        return _NC_CACHE["runner"]

    import jax
    import numpy as _np
    from jax.sharding import Mesh, PartitionSpec
    from jax.experimental.shard_map import shard_map
    import concourse.mybir as mybir
    from concourse import bass2jax

    nc = _NC_CACHE["nc"]
    bass2jax.install_neuronx_cc_hook()
    assert nc.dbg_addr is None

    partition_name = (
        nc.partition_id_tensor.name if nc.partition_id_tensor else None
    )
    in_names = []
    out_names = []
    out_avals = []
    zero_shapes = []
    for alloc in nc.m.functions[0].allocations:
        if not isinstance(alloc, mybir.MemoryLocationSet):
            continue
        name = alloc.memorylocations[0].name
        if alloc.kind == "ExternalInput":
            if name != partition_name:
                in_names.append(name)
        elif alloc.kind == "ExternalOutput":
            shape = tuple(alloc.tensor_shape)
            dtype = mybir.dt.np(alloc.dtype)
            out_names.append(name)
            out_avals.append(jax.core.ShapedArray(shape, dtype))
            zero_shapes.append((shape, dtype))
    n_params = len(in_names)
    all_names = list(in_names) + list(out_names)
    if partition_name is not None:
        all_names.append(partition_name)
    donate = tuple(range(n_params, n_params + len(out_names)))

    def _body(*args):
        operands = list(args)
        if partition_name is not None:
            operands.append(bass2jax.partition_id_tensor())
        outs = bass2jax._bass_exec_p.bind(
            *operands,
            out_avals=tuple(out_avals),
            in_names=tuple(all_names),
            out_names=tuple(out_names),
            lowering_input_output_aliases=(),
            sim_require_finite=True,
            sim_require_nnan=True,
            nc=nc,
        )
        return tuple(outs)

    devices = jax.devices()[:8]
    mesh = Mesh(_np.asarray(devices), ("core",))
    in_specs = (PartitionSpec("core"),) * (n_params + len(out_names))
    out_specs = (PartitionSpec("core"),) * len(out_names)
    sharded = jax.jit(
        shard_map(
            _body, mesh=mesh, in_specs=in_specs, out_specs=out_specs,
            check_rep=False,
        ),
        donate_argnums=donate,
        keep_unused=True,
    )
    runner = (sharded, in_names, out_names, out_avals, zero_shapes)
    _NC_CACHE["runner"] = runner
    return runner


def _run_cached(in_maps):
    import numpy as _np

    sharded, in_names, out_names, out_avals, zero_shapes = _get_runner()
    concat_in = [
        _np.concatenate([_np.asarray(m[name]) for m in in_maps], axis=0)
        for name in in_names
    ]
    concat_zeros = [
        _np.zeros((8 * s[0], *s[1:]), dt) for s, dt in zero_shapes
    ]
    out_arrs = sharded(*concat_in, *concat_zeros)
    return [
        {
            name: _np.asarray(out_arrs[i]).reshape(8, *out_avals[i].shape)[c]
            for i, name in enumerate(out_names)
        }
        for c in range(8)
    ]


def kernel(x, params):
    in_maps, SD = make_in_maps(x, params)

    if "nc" not in _NC_CACHE:
        _NC_CACHE["nc"] = _build_nc()

    try:
        outs = _run_cached(in_maps)
    except Exception:
        from concourse.bass_utils import run_bass_kernel_spmd

        _NC_CACHE.pop("runner", None)
        res = run_bass_kernel_spmd(
            _NC_CACHE["nc"], in_maps, core_ids=list(range(8))
        )
        outs = res.results

    total = np.zeros(N, np.float64)
    for d in range(8):
        total += SD[d].astype(np.float64) * np.asarray(
            outs[d]["out"], np.float64
        ).sum(axis=1)
    return total.astype(np.float32)
